# revision 2
# baseline (speedup 1.0000x reference)
"""Partial-FC conv classifier kernel for 8 TRN2 NeuronCores.

Problem (hardcoded shapes): x [512, 512, 7, 7] f32, labels [512] i64,
weight [85742, 512, 1, 1] f32, bias [85742] f32.
reference: labels_unique = unique(labels, size=512, fill=0); w_sub =
weight[labels_unique]; logits = conv1x1(x, w_sub) + b_sub -> [512, 512, 7, 7].

Strategy: the unique-label gather is host-side data staging (it selects
512 rows / 1MB out of the 176MB table). The conv1x1 is a matmul
  out[u, (b,s)] = sum_c w_sub[u, c] * x[b, c, s].
Data-parallel over batch: core i computes batches [64*i, 64*(i+1)) with the
gathered weight replicated. Per core: [512x512] @ [512x3136] in fp16 with
fp32 PSUM accumulation (fp8 was measured at 4e-2 rel err -- fails the 2e-2
gate -- so fp16's 20.9us PE floor is the real compute roofline here).

Schedule (from trace analysis of the 40us baseline): the framework preamble
ends ~6.3us; after that the limiter was serialized DMA descriptor generation
on the Sync engine (~0.65us per DMA_DIRECT2D) plus a w-gated first matmul,
which pushed real compute to 15.5us and let the PE HAM clock-gate drop back
to half rate. Here:
  - w is split into 4 per-k DMAs and x into a 448-col head chunk (per-k)
    plus 3x 896-col body chunks (per-k), issued round-robin on BOTH HWDGE
    rings (Sync + Scalar) so the first matmul is gated on just ~230KB.
  - cols 0:1792 run k-OUTER (4 open PSUM banks per 448-col section): each
    matmul stage is gated only on its own (w_k, x_k) slice, so compute
    starts at ~7.4us and stays continuous (keeps the HAM clock at full
    rate, no warm-up gap).
  - cols 1792:3136 run k-inner/m-outer (x resident by then); the last
    section drains per-m so only ~115KB of output remains after the
    final matmul.
  - PSUM eviction (+bias, fp32->fp16) alternates Vector/Scalar; output
    pieces are sized to overlap compute and issued from whatever engine
    produced them.
"""

import numpy as np

import concourse.bass as bass  # noqa: F401  (registers types)
import concourse.mybir as mybir
import concourse.tile as tile
from concourse import bacc
from concourse.bass_utils import run_bass_kernel_spmd

N_CORES = 8
B = 512          # batch
C = 512          # channels (contraction)
HW = 49          # 7*7 spatial
U = 512          # unique labels (all distinct by construction)
B_LOC = B // N_CORES      # 64 batches per core
N_LOC = B_LOC * HW        # 3136 moving-dim columns per core
KT = C // 128             # 4 contraction tiles
MT = U // 128             # 4 output-partition tiles
SEC = 448                 # section width (one PSUM bank at fp32)
NSEC = N_LOC // SEC       # 7 sections: A, S1..S6
A_END = SEC               # phase A cols [0, 448)
B1 = (448, 1344)          # body x chunks (per-k DMAs of 896 cols)
B2 = (1344, 2240)
B3 = (2240, 3136)
KO_SECS = 4               # sections 0..3 (cols 0:1792) run k-outer
N_WARM = 4                # 256-col warm-up matmuls bridging DMA latency

F32 = mybir.dt.float32
F16 = mybir.dt.float16

_MODULE = None


def _build_module():
    nc = bacc.Bacc("TRN2", target_bir_lowering=False, debug=False)
    # layouts are pre-swizzled on the host so every DMA is a plain
    # partition-major copy with large contiguous per-partition runs
    xT = nc.dram_tensor("xT", [KT, 128, N_LOC], F16, kind="ExternalInput").ap()
    wT = nc.dram_tensor("wT", [KT, 128, U], F16, kind="ExternalInput").ap()
    bs = nc.dram_tensor("bs", [128, MT], F32, kind="ExternalInput").ap()
    out = nc.dram_tensor("out", [U, N_LOC], F16, kind="ExternalOutput").ap()

    with tile.TileContext(nc) as tc:
        with (
            tc.tile_pool(name="wpool", bufs=KT) as wpool,
            tc.tile_pool(name="bpool", bufs=1) as bpool,
            tc.tile_pool(name="scr", bufs=1) as scr,
            tc.tile_pool(name="xapool", bufs=KT) as xapool,
            tc.tile_pool(name="xbpool", bufs=3 * KT) as xbpool,
            tc.tile_pool(name="opool", bufs=MT) as opool,
            tc.tile_pool(name="psum", bufs=8, space="PSUM") as psum,
        ):
            # ---- input DMA issue: round-robin across both HWDGE rings so
            # descriptor generation never serializes the critical path.
            w_sb = [wpool.tile([128, U], F16, tag="w", name=f"w_{k}")
                    for k in range(KT)]
            xa_sb = [xapool.tile([128, SEC], F16, tag="xa", name=f"xa_{k}")
                     for k in range(KT)]
            xb_sb = [[xbpool.tile([128, c1 - c0], F16, tag="xb",
                                  name=f"xb_{bi}_{k}")
                      for k in range(KT)]
                     for bi, (c0, c1) in enumerate((B1, B2, B3))]
            b_sb = bpool.tile([128, MT], F32)

            # Sync ring: w_k0, w_k1, bias, w_k2, w_k3, then B1 per-k
            # Scalar ring: xa per-k, then B2 per-k (B3 after evictions start)
            nc.sync.dma_start(w_sb[0][:], wT[0])
            nc.scalar.dma_start(xa_sb[0][:], xT[0][:, 0:A_END])
            nc.sync.dma_start(w_sb[1][:], wT[1])
            nc.scalar.dma_start(xa_sb[1][:], xT[1][:, 0:A_END])
            nc.sync.dma_start(b_sb[:], bs[:])
            nc.scalar.dma_start(xa_sb[2][:], xT[2][:, 0:A_END])
            nc.sync.dma_start(w_sb[2][:], wT[2])
            nc.scalar.dma_start(xa_sb[3][:], xT[3][:, 0:A_END])
            nc.sync.dma_start(w_sb[3][:], wT[3])
            for k in range(KT):
                nc.sync.dma_start(xb_sb[0][k][:], xT[k][:, B1[0]:B1[1]])
                nc.scalar.dma_start(xb_sb[1][k][:], xT[k][:, B2[0]:B2[1]])

            # Warm-ups: dependency-free matmuls on zeroed scratch keep the
            # PE busy (and the HAM clock-gate ramping) while x streams in.
            scr_sb = scr.tile([128, 256], F16)
            nc.gpsimd.memset(scr_sb[:], 0.0)
            for i in range(N_WARM):
                ps_warm = psum.tile([128, 256], F32, tag="ps", name=f"warm_{i}")
                nc.tensor.matmul(
                    ps_warm[:], scr_sb[:, :128], scr_sb[:],
                    start=True, stop=True,
                )

            # Output staging: one full row-block per m-tile
            o_sb = [opool.tile([128, N_LOC], F16, tag="o", name=f"o_{m}")
                    for m in range(MT)]

            def x_slice(k, c0, c1):
                """SBUF view of x k-tile columns [c0, c1)."""
                if c1 <= A_END:
                    return xa_sb[k][:, c0:c1]
                for bi, (b0, b1) in enumerate((B1, B2, B3)):
                    if c0 >= b0 and c1 <= b1:
                        return xb_sb[bi][k][:, c0 - b0:c1 - b0]
                raise AssertionError((c0, c1))

            def evict(ps, m, c0, c1, eng):
                dst = o_sb[m][:, c0:c1]
                if eng == "s":
                    nc.scalar.activation(
                        dst, ps[:], mybir.ActivationFunctionType.Identity,
                        bias=b_sb[:, m:m + 1],
                    )
                else:
                    nc.vector.tensor_scalar_add(dst, ps[:], b_sb[:, m:m + 1])

            # ---- sections 0..3 (cols 0:1792): k-outer so each matmul stage
            # is gated on one (w_k, x_k) slice; 4 PSUM banks per section.
            for s in range(KO_SECS):
                c0, c1 = s * SEC, (s + 1) * SEC
                ps_s = [psum.tile([128, SEC], F32, tag="ps",
                                  name=f"ps_{s}_{m}") for m in range(MT)]
                for k in range(KT):
                    xs = x_slice(k, c0, c1)
                    for m in range(MT):
                        nc.tensor.matmul(
                            ps_s[m][:],
                            w_sb[k][:, m * 128:(m + 1) * 128],
                            xs,
                            start=(k == 0),
                            stop=(k == KT - 1),
                        )
                if s == 0:
                    # both rings are now past their startup-critical issues;
                    # stream the last body chunk on Scalar
                    for k in range(KT):
                        nc.scalar.dma_start(xb_sb[2][k][:],
                                            xT[k][:, B3[0]:B3[1]])
                    for m in range(MT):  # phase A evictions: all on Vector
                        evict(ps_s[m], m, c0, c1, "v")
                else:
                    for m in range(MT):
                        evict(ps_s[m], m, c0, c1, "v" if m % 2 == 0 else "s")
                if s == 3:
                    # cols 0:1792 of each row block complete -> first piece
                    for m in range(MT):
                        nc.sync.dma_start(out[m * 128:(m + 1) * 128, 0:1792],
                                          o_sb[m][:, 0:1792])

            # ---- sections 4..6 (cols 1792:3136): x is resident; k-inner
            # per m so outputs drain per-m with a short tail.
            for s in range(KO_SECS, NSEC):
                c0, c1 = s * SEC, (s + 1) * SEC
                for m in range(MT):
                    ps = psum.tile([128, SEC], F32, tag="ps",
                                   name=f"ps_{s}_{m}")
                    for k in range(KT):
                        nc.tensor.matmul(
                            ps[:],
                            w_sb[k][:, m * 128:(m + 1) * 128],
                            x_slice(k, c0, c1),
                            start=(k == 0),
                            stop=(k == KT - 1),
                        )
                    eng = "v" if m % 2 == 0 else "s"
                    evict(ps, m, c0, c1, eng)
                    if s == NSEC - 1:
                        # last section: drain per-m from the evicting engine
                        # (Vector can't issue DMAs; route its rows via Sync)
                        dma_eng = nc.scalar if eng == "s" else nc.sync
                        dma_eng.dma_start(
                            out[m * 128:(m + 1) * 128, 2240:3136],
                            o_sb[m][:, 2240:3136],
                        )
                if s == NSEC - 2:
                    # cols 1792:2688 complete per row block -> second piece
                    for m in range(MT):
                        nc.sync.dma_start(
                            out[m * 128:(m + 1) * 128, 1792:2240],
                            o_sb[m][:, 1792:2240],
                        )

    nc.compile()
    return nc


def _get_module():
    global _MODULE
    if _MODULE is None:
        _MODULE = _build_module()
    return _MODULE


def _prep_inputs(x, labels, weight, bias):
    x = np.asarray(x)
    labels = np.asarray(labels)
    weight = np.asarray(weight)
    bias = np.asarray(bias, dtype=np.float32)

    # jnp.unique(labels, size=B, fill_value=0): sorted unique, padded with 0.
    u = np.unique(labels)
    if u.size < U:
        u = np.concatenate([u, np.zeros(U - u.size, dtype=u.dtype)])
    u = u[:U]

    w_sub = weight.reshape(weight.shape[0], C)[u]                    # [U, C]
    # wT[t, p, m] = w_sub[m, t*128+p]
    wT = np.ascontiguousarray(w_sub.T.astype(np.float16).reshape(KT, 128, U))
    b_sub = np.ascontiguousarray(bias[u].reshape(MT, 128).T)         # [128, MT]

    x16 = x.reshape(B, C, HW).astype(np.float16)
    in_maps = []
    for i in range(N_CORES):
        xi = x16[i * B_LOC:(i + 1) * B_LOC]
        # c = t*128+p, col = b*49+s
        xt = np.ascontiguousarray(
            xi.transpose(1, 0, 2).reshape(KT, 128, N_LOC)
        )
        in_maps.append({"xT": xt, "wT": wT, "bs": b_sub})
    return in_maps


def _assemble_output(results):
    parts = []
    for i in range(N_CORES):
        oi = np.asarray(results[i]["out"]).astype(np.float32)  # [U, N_LOC]
        parts.append(
            np.ascontiguousarray(
                oi.reshape(U, B_LOC, HW).transpose(1, 0, 2)
            ).reshape(B_LOC, U, 7, 7)
        )
    return np.concatenate(parts, axis=0)


def run(x, labels, weight, bias, trace=False):
    in_maps = _prep_inputs(x, labels, weight, bias)
    nc = _get_module()
    res = run_bass_kernel_spmd(
        nc, in_maps, core_ids=list(range(N_CORES)), trace=trace
    )
    return _assemble_output(res.results), res


def kernel(x, labels, weight, bias):
    out, _ = run(x, labels, weight, bias, trace=False)
    return out
